# revision 4
# baseline (speedup 1.0000x reference)
"""GraphConv 2-layer GNN on 8 Trainium2 NeuronCores.

Strategy (per sharding hint): destination nodes are partitioned across the 8
cores (12500 each).  The host shards/permutes edge payloads into a
column-major, degree-sorted window layout; each core streams its edge
payloads and does the entire segment-sum reduction (exact fp32 PSUM
accumulation via identity matmuls), the dense W_rel/W_root matmuls, bias,
relu and log-softmax on device.  Two SPMD launches (one per GraphConv
layer) with a host allgather of the hidden state between them.
"""
import sys
sys.path.insert(0, "/opt/trn_rl_repo")
import numpy as np
import ml_dtypes

import concourse.bacc as bacc
import concourse.mybir as mybir
import concourse.tile as tile
from concourse.bass_utils import run_bass_kernel_spmd
from concourse.masks import make_identity

BF16 = ml_dtypes.bfloat16
N, E, F, H, C = 100000, 1600000, 128, 128, 40
NCORES = 8
OWN = N // NCORES          # 12500 dst nodes per core
P = 128
NWIN = (OWN + P - 1) // P  # 98 windows
OWNP = NWIN * P            # 12544 padded

BF = mybir.dt.bfloat16
F32 = mybir.dt.float32

_cache = {}


def _prep_graph(edge_index):
    """Host-side shard/permute plan. Returns per-core canonical node orders,
    the common window depth profile Dw, slot offsets, and per-core edge
    index matrices idxmat [L, 128] (src global id or -1)."""
    src = np.asarray(edge_index[0], dtype=np.int64)
    dst = np.asarray(edge_index[1], dtype=np.int64)
    deg = np.bincount(dst, minlength=N)
    orders = []
    for c in range(NCORES):
        ids = np.arange(c * OWN, (c + 1) * OWN)
        orders.append(ids[np.argsort(-deg[ids], kind="stable")])
    degs_sorted = np.stack([deg[o] for o in orders])  # [8, OWN]
    pad = np.zeros((NCORES, OWNP - OWN), np.int64)
    degs_sorted = np.concatenate([degs_sorted, pad], axis=1)
    Dw = []
    for w in range(NWIN):
        seg = degs_sorted[:, w * P:(w + 1) * P]
        Dw.append(max(1, int(seg.max())))
    offs = np.zeros(NWIN + 1, np.int64)
    offs[1:] = np.cumsum(Dw)
    L = int(offs[-1])

    core = dst // OWN
    idxmats = []
    for c in range(NCORES):
        rank_of = np.empty(OWN, np.int64)
        rank_of[orders[c] - c * OWN] = np.arange(OWN)
        m = core == c
        s_c, d_c = src[m], dst[m]
        r = rank_of[d_c - c * OWN]
        order = np.argsort(r, kind="stable")
        r_s = r[order]
        # occurrence index of each edge within its dst group
        first = np.searchsorted(r_s, r_s)
        j = np.arange(len(r_s)) - first
        win = r_s // P
        lane = r_s % P
        idxmat = np.full((L, P), -1, np.int64)
        idxmat[offs[win] + j, lane] = s_c[order]
        idxmats.append(idxmat)
    return orders, Dw, offs, idxmats, L


def _payload(table_bf16, idxmat, Dw, offs):
    """Build the flat per-core edge payload: per window, [P, Dw[w]*F] bf16,
    partition-contiguous."""
    L = idxmat.shape[0]
    pay = np.zeros((L, P, F), BF16)
    valid = idxmat >= 0
    pay[valid] = table_bf16[idxmat[valid]]
    chunks = [
        np.ascontiguousarray(pay[offs[w]:offs[w + 1]].transpose(1, 0, 2)).reshape(-1, F)
        for w in range(len(Dw))
    ]
    return np.concatenate(chunks, axis=0)


def _build(layer, Dw, offs, R=1):
    """Build the SPMD Bass program for one GraphConv layer."""
    FO = H if layer == 1 else C
    L = int(offs[-1])
    nc = bacc.Bacc()
    xe = nc.declare_dram_parameter("xe", [L * P, F], BF, isOutput=False)
    xr = nc.declare_dram_parameter("xr", [OWNP, F], BF, isOutput=False)
    wrel = nc.declare_dram_parameter("wrel", [F, FO], BF, isOutput=False)
    wroot = nc.declare_dram_parameter("wroot", [F, FO], BF, isOutput=False)
    bias = nc.declare_dram_parameter("bias", [P, 1], F32, isOutput=False)
    if layer == 1:
        out = nc.declare_dram_parameter("out", [OWNP, FO], BF, isOutput=True)
    else:
        out = nc.declare_dram_parameter("out", [OWNP, FO], F32, isOutput=True)

    with tile.TileContext(nc) as tc:
        with (
            tc.tile_pool(name="const", bufs=1) as cpool,
            tc.tile_pool(name="stream", bufs=3) as spool,
            tc.tile_pool(name="work", bufs=3) as epool,
            tc.tile_pool(name="small", bufs=4) as mpool,
            tc.tile_pool(name="ps", bufs=2, space="PSUM") as ppool,
        ):
            ident = cpool.tile([P, P], BF)
            make_identity(nc, ident[:])
            wrel_t = cpool.tile([F, FO], BF)
            nc.sync.dma_start(out=wrel_t[:], in_=wrel[:])
            wroot_t = cpool.tile([F, FO], BF)
            nc.sync.dma_start(out=wroot_t[:], in_=wroot[:])
            bias_t = cpool.tile([P, 1], F32)
            nc.sync.dma_start(out=bias_t[:], in_=bias[:])

            def body(_iv=None):
                for w in range(NWIN):
                    D = int(Dw[w])
                    base = int(offs[w]) * P
                    st = spool.tile([P, D * F], BF, tag="stream")
                    nc.sync.dma_start(
                        out=st[:],
                        in_=xe[base:base + P * D, :].rearrange(
                            "(p d) f -> p (d f)", p=P),
                    )
                    agg_ps = ppool.tile([P, P], F32, tag="agg")
                    for j in range(D):
                        nc.tensor.matmul(
                            out=agg_ps[:],
                            lhsT=ident[:],
                            rhs=st[:, j * F:(j + 1) * F],
                            start=(j == 0),
                            stop=(j == D - 1),
                        )
                    agg_sb = epool.tile([P, P], BF, tag="aggsb")
                    nc.vector.tensor_copy(out=agg_sb[:], in_=agg_ps[:])
                    aggT_ps = ppool.tile([P, P], BF, tag="tr")
                    nc.tensor.transpose(out=aggT_ps[:], in_=agg_sb[:], identity=ident[:])
                    aggT_sb = epool.tile([P, P], BF, tag="aggT")
                    nc.vector.tensor_copy(out=aggT_sb[:], in_=aggT_ps[:])
                    xtw = epool.tile([P, P], BF, tag="xtw")
                    nc.sync.dma_start(
                        out=xtw[:], in_=xr[w * P:(w + 1) * P, :], transpose=True
                    )
                    ht_ps = ppool.tile([P, P], F32, tag="ht")
                    nc.tensor.matmul(out=ht_ps[:FO, :], lhsT=wrel_t[:], rhs=aggT_sb[:],
                                     start=True, stop=False)
                    nc.tensor.matmul(out=ht_ps[:FO, :], lhsT=wroot_t[:], rhs=xtw[:],
                                     start=False, stop=True)
                    if layer == 1:
                        ht_sb = epool.tile([P, P], BF, tag="htsb")
                        nc.scalar.activation(
                            out=ht_sb[:], in_=ht_ps[:],
                            func=mybir.ActivationFunctionType.Relu,
                            bias=bias_t[:, :1], scale=1.0,
                        )
                        h_ps = ppool.tile([P, P], BF, tag="tr")
                        nc.tensor.transpose(out=h_ps[:], in_=ht_sb[:], identity=ident[:])
                        h_sb = epool.tile([P, P], BF, tag="hsb")
                        nc.vector.tensor_copy(out=h_sb[:], in_=h_ps[:])
                        nc.sync.dma_start(out=out[w * P:(w + 1) * P, :], in_=h_sb[:])
                    else:
                        ot_sb = epool.tile([P, P], BF, tag="otsb")
                        nc.vector.tensor_scalar_add(
                            out=ot_sb[:FO, :], in0=ht_ps[:FO, :],
                            scalar1=bias_t[:FO, :1],
                        )
                        o_ps = ppool.tile([P, P], BF, tag="tr")
                        nc.tensor.transpose(
                            out=o_ps[:, :FO], in_=ot_sb[:FO, :],
                            identity=ident[:FO, :FO],
                        )
                        t_sb = epool.tile([P, FO], F32, tag="tsb")
                        nc.vector.tensor_copy(out=t_sb[:], in_=o_ps[:, :FO])
                        mx = mpool.tile([P, 1], F32, tag="mx")
                        nc.vector.reduce_max(out=mx[:], in_=t_sb[:],
                                             axis=mybir.AxisListType.X)
                        negm = mpool.tile([P, 1], F32, tag="negm")
                        nc.scalar.mul(out=negm[:], in_=mx[:], mul=-1.0)
                        ex = epool.tile([P, FO], F32, tag="ex")
                        nc.scalar.activation(
                            out=ex[:], in_=t_sb[:],
                            func=mybir.ActivationFunctionType.Exp,
                            bias=negm[:, :1], scale=1.0,
                        )
                        sm = mpool.tile([P, 1], F32, tag="sm")
                        nc.vector.reduce_sum(out=sm[:], in_=ex[:],
                                             axis=mybir.AxisListType.X)
                        ls = mpool.tile([P, 1], F32, tag="ls")
                        nc.scalar.activation(
                            out=ls[:], in_=sm[:],
                            func=mybir.ActivationFunctionType.Ln,
                        )
                        tot = mpool.tile([P, 1], F32, tag="tot")
                        nc.vector.tensor_add(out=tot[:], in0=mx[:], in1=ls[:])
                        res = epool.tile([P, FO], F32, tag="res")
                        nc.vector.tensor_scalar_sub(out=res[:], in0=t_sb[:],
                                                    scalar1=tot[:, :1])
                        nc.sync.dma_start(out=out[w * P:(w + 1) * P, :], in_=res[:])

            if R > 1:
                with tc.For_i(0, R, 1):
                    body()
            else:
                body()

    nc.finalize()
    return nc


def _layer_inputs(table_bf16, orders, Dw, offs, idxmats, w_rel, w_root, b):
    FO = w_rel.shape[0]
    wrelT = np.ascontiguousarray(w_rel.T).astype(BF16)
    wrootT = np.ascontiguousarray(w_root.T).astype(BF16)
    bias = np.zeros((P, 1), np.float32)
    bias[:FO, 0] = b
    in_maps = []
    for c in range(NCORES):
        xe = _payload(table_bf16, idxmats[c], Dw, offs)
        xr = np.zeros((OWNP, F), BF16)
        xr[:OWN] = table_bf16[orders[c]]
        in_maps.append({"xe": xe, "xr": xr, "wrel": wrelT, "wroot": wrootT,
                        "bias": bias})
    return in_maps


def kernel(x, edge_index, W1_rel, b1, W1_root, W2_rel, b2, W2_root, _R=1,
           _return_nc=False):
    x = np.asarray(x, np.float32)
    key = id(edge_index)
    orders, Dw, offs, idxmats, L = _prep_graph(edge_index)

    nc1 = _build(1, Dw, offs, R=_R)
    nc2 = _build(2, Dw, offs, R=_R)

    xbf = x.astype(BF16)
    in1 = _layer_inputs(xbf, orders, Dw, offs, idxmats,
                        np.asarray(W1_rel, np.float32), np.asarray(W1_root, np.float32),
                        np.asarray(b1, np.float32))
    res1 = run_bass_kernel_spmd(nc1, in1, list(range(NCORES)))
    h_glob = np.zeros((N, F), BF16)
    for c in range(NCORES):
        h_glob[orders[c]] = res1.results[c]["out"][:OWN]

    in2 = _layer_inputs(h_glob, orders, Dw, offs, idxmats,
                        np.asarray(W2_rel, np.float32), np.asarray(W2_root, np.float32),
                        np.asarray(b2, np.float32))
    res2 = run_bass_kernel_spmd(nc2, in2, list(range(NCORES)))
    out = np.zeros((N, C), np.float32)
    for c in range(NCORES):
        out[orders[c]] = res2.results[c]["out"][:OWN]
    return out
